# revision 25
# baseline (speedup 1.0000x reference)
"""GAT 2-layer encoder on 8 Trainium2 NeuronCores.

Reference computation: layer 1 = GAT conv over edge_index[:, :500] (weights W1),
layer 2 = GAT conv over edge_index[:, 500:] (weights W2).

Strategy:
  - Layer-1 output x1 differs from the default row b1 only on the <=500
    distinct dsts of the first 500 edges ("specials").  In layer 2 every edge
    whose src is non-special carries the identical feature row x1_def = b1 and
    (for a fixed dst d) the identical score c_d = leaky(sigma_def + delta_d),
    so the (deg_d - k_d) default edges of d collapse into ONE closed-form
    softmax term with weight ndef*exp(c_d): the term's feature row b1@W2 is
    folded into the output matmul as an extra stationary row scaled by a
    per-dst weight wdef.
  - Only dsts with k_d > 0 (or deg_d = 0) need device processing: ~8.5k of
    the 1.6M edges.  Every other dst's output row equals the default output
    row [b1|1] @ [W2;b2], broadcast-written from SBUF in two large DMAs that
    overlap the compute.
  - Layer 1 aggregates raw x rows (linearity: sum(alpha*(x@W1)) =
    (sum(alpha*x))@W1), so edge features are fetched with two multi-row
    indirect DMAs straight from x -- no intermediate feature table.
  - Sharding: dst-range partition across 8 cores (no collectives; layer 1 +
    table build replicated on every core, it is tiny).  Host side does index
    computation only (degree counts, grid layout, log of integer counts).
"""

import sys

sys.path.insert(0, "/opt/trn_rl_repo")

from contextlib import ExitStack

import numpy as np

import concourse.bacc as bacc
import concourse.bass as bass
import concourse.mybir as mybir
import concourse.tile as tile
from concourse.bass_utils import run_bass_kernel_spmd
from concourse.masks import make_identity

F32 = mybir.dt.float32
I16 = mybir.dt.int16
I32 = mybir.dt.int32
AF = mybir.ActivationFunctionType
OP = mybir.AluOpType

N = 100000
D = 64
NCORES = 8
NPC = N // NCORES          # dst nodes per core
P = 128
NSPLIT = 500               # first 500 edges -> layer 1
NEG_SLOPE = 0.2
EPS = 1e-16
BIG = 200.0                # score shift so padded slots underflow exp to 0.0
NPCPAD = ((NPC + P - 1) // P) * P


def _wrap16(flat):
    """int16 stream [n] (n%16==0) -> dma_gather idx tile [128, n//16]."""
    w = flat.reshape(-1, 16).T
    return np.ascontiguousarray(np.tile(w, (8, 1)).astype(np.int16))


def _groups(L):
    """Contiguous runs of equal L -> [{b0, B, L, slot_off}]."""
    slot_base = np.concatenate([[0], np.cumsum(L)])[:-1].astype(np.int64)
    out = []
    b = 0
    while b < len(L):
        b0 = b
        while b < len(L) and L[b] == L[b0]:
            b += 1
        out.append({"b0": b0, "B": b - b0, "L": L[b0],
                    "slot_off": int(slot_base[b0])})
    return out, slot_base


def prep(inputs):
    """Host-side index prep (pure index computation, no feature values)."""
    ei = np.asarray(inputs["edge_index"])
    src = ei[0].astype(np.int64)
    dst = ei[1].astype(np.int64)
    s1, d1 = src[:NSPLIT], dst[:NSPLIT]
    s2, d2 = src[NSPLIT:], dst[NSPLIT:]

    # ---- layer 1 grid over the K specials (+1 pad position -> default row) --
    specials, deg1 = np.unique(d1, return_counts=True)
    K = len(specials)
    order1 = np.argsort(-deg1, kind="stable")
    spec_by_pos = specials[order1]
    nblk1 = (K + 1 + P - 1) // P
    npos1 = nblk1 * P
    deg1s = np.zeros(npos1, np.int64)
    deg1s[:K] = deg1[order1]
    L1 = [max(int(deg1s[b * P:(b + 1) * P].max()), 1) for b in range(nblk1)]
    groups1, slot_base1 = _groups(L1)
    S1 = int(sum(L1))

    # table row of grid position q = (q%P)*nblk1 + q//P (partition-major so
    # the whole table is one DMA from a [128, nblk1, 128] SBUF tile)
    qarr = np.arange(K)
    defrow = (K % P) * nblk1 + K // P
    rowmap = np.full(N, defrow, np.int64)
    rowmap[spec_by_pos] = (qarr % P) * nblk1 + qarr // P
    TABR = nblk1 * P

    # layer-1 edge slots (values = src NODE ids for indirect x gather)
    rank1 = np.empty(K, np.int64)
    rank1[order1] = np.arange(K)
    d1pos = rank1[np.searchsorted(specials, d1)]
    pe = np.argsort(d1pos, kind="stable")
    pos_s = d1pos[pe]
    val_s = s1[pe]
    start_of_pos = np.searchsorted(pos_s, np.arange(npos1))
    kk = np.arange(len(pos_s)) - start_of_pos[pos_s]
    flat = (slot_base1[pos_s // P] + kk) * P + (pos_s % P)
    sidx1 = np.zeros(S1 * P, np.int32)
    sidx1[flat] = val_s
    l1m = np.zeros(S1 * P, np.float32)
    l1m[flat] = 1.0
    sidx1 = np.ascontiguousarray(sidx1.reshape(S1, P).T)
    l1_mask = np.ascontiguousarray(l1m.reshape(S1, P).T)
    dn = np.zeros(npos1, np.int64)
    dn[:K] = spec_by_pos
    didx1 = np.ascontiguousarray(dn.reshape(nblk1, P).T.astype(np.int32))
    dp = np.zeros(npos1, np.float32)
    dp[:K] = 1.0
    l1_degpos = np.ascontiguousarray(dp.reshape(nblk1, P).T)

    # ---- layer 2: affected dsts only (k>0 special in-edges, or deg==0) ----
    core_dat = []
    for c in range(NCORES):
        sel = (d2 >= c * NPC) & (d2 < (c + 1) * NPC)
        dl = d2[sel] - c * NPC
        sl = s2[sel]
        deg = np.bincount(dl, minlength=NPC)
        spm = rowmap[sl] != defrow
        dls = dl[spm]
        sls = sl[spm]
        kcnt = np.bincount(dls, minlength=NPC)
        aff = (kcnt > 0) | (deg == 0)
        A = np.nonzero(aff)[0]
        is_sp = rowmap[c * NPC + A] != defrow
        ordA = np.lexsort((-kcnt[A], np.logical_not(is_sp)))
        core_dat.append({"deg": deg, "k": kcnt, "A_sorted": A[ordA],
                         "nspec": int(is_sp.sum()), "dls": dls, "sls": sls,
                         "aff": aff})

    nAmax = max(len(cd["A_sorted"]) for cd in core_dat)
    nblkA = max(1, (nAmax + P - 1) // P)
    nApad = nblkA * P
    nSmax = max(cd["nspec"] for cd in core_dat)
    nSblk = min(nblkA, max(1, (nSmax + P - 1) // P))
    kprof = np.zeros(nApad, np.int64)
    for cd in core_dat:
        kk2 = cd["k"][cd["A_sorted"]]
        kprof[:len(kk2)] = np.maximum(kprof[:len(kk2)], kk2)
    L2 = [max(int(kprof[b * P:(b + 1) * P].max()), 1) for b in range(nblkA)]
    groups2, slot_base2 = _groups(L2)
    S2 = int(sum(L2))
    NROWS = nApad + NPCPAD
    Wd = NPCPAD // P

    cores = []
    for c, cd in enumerate(core_dat):
        A_sorted = cd["A_sorted"]
        nA = len(A_sorted)
        rankA = np.full(NPC, -1, np.int64)
        rankA[A_sorted] = np.arange(nA)
        pos = rankA[cd["dls"]]
        pe2 = np.argsort(pos, kind="stable")
        pos_s2 = pos[pe2]
        val2 = rowmap[cd["sls"][pe2]]
        start2 = np.searchsorted(pos_s2, np.arange(nApad))
        kk2 = np.arange(len(pos_s2)) - start2[pos_s2]
        sglob = slot_base2[pos_s2 // P] + kk2
        flat2 = sglob * P + (pos_s2 % P)
        eidxflat = np.full(S2 * P, defrow, np.int16)
        eidxflat[flat2] = val2
        maskflat = np.zeros(S2 * P, np.float32)
        maskflat[flat2] = 1.0
        mask2 = np.ascontiguousarray(maskflat.reshape(S2, P).T)
        eidx = _wrap16(eidxflat)
        # one-hot permutation [128, nSblk*nblk1*128]: for block-0.. special
        # positions p, PERM[sb][w][p_src, p] = 1 iff the dst's table row is
        # p_src*nblk1 + w  (non-special dsts select the default row)
        dvals = np.full(nSblk * P, defrow, np.int64)
        nn = min(nA, nSblk * P)
        dvals[:nn] = rowmap[c * NPC + A_sorted[:nn]]
        perm = np.zeros((nSblk, nblk1, P, P), np.float32)
        qq = np.arange(nSblk * P)
        perm[qq // P, dvals % nblk1, dvals // nblk1, qq % P] = 1.0
        perm = np.ascontiguousarray(
            perm.transpose(2, 0, 1, 3).reshape(P, nSblk * nblk1 * P))
        # per-position arrays [128, nblkA]
        degq = np.zeros(nApad, np.float32)
        pmq = np.zeros(nApad, np.float32)
        lnnq = np.zeros(nApad, np.float32)
        kq = cd["k"][A_sorted]
        ndef = cd["deg"][A_sorted] - kq
        degq[:nA] = (cd["deg"][A_sorted] > 0)
        pmq[:nA] = (ndef > 0)
        lnnq[:nA] = np.log(np.maximum(ndef, 1).astype(np.float64))
        degpos2 = np.ascontiguousarray(degq.reshape(nblkA, P).T)
        pm2 = np.ascontiguousarray(pmq.reshape(nblkA, P).T)
        lnn2 = np.ascontiguousarray(lnnq.reshape(nblkA, P).T)
        fpack = np.ascontiguousarray(np.concatenate(
            [l1_mask, l1_degpos, mask2, degpos2, pm2, lnn2, perm], axis=1))
        i16pack = eidx
        cores.append({"fpack": fpack, "i16pack": i16pack,
                      "A_sorted": A_sorted,
                      "non": np.nonzero(~cd["aff"])[0]})

    i32pack = np.ascontiguousarray(np.concatenate([sidx1, didx1], axis=1))

    meta = {
        "K": K, "nblk1": nblk1, "S1": S1, "groups1": groups1,
        "S2": S2, "nblkA": nblkA, "nSblk": nSblk, "groups2": groups2,
        "nApad": nApad, "NROWS": NROWS, "Wd": Wd, "TABR": TABR,
        "defrow": defrow,
        "FW": S1 + nblk1 + S2 + 3 * nblkA + nSblk * nblk1 * P,
        "IW32": S1 + nblk1, "IW16": 8 * S2,
    }
    l1 = {"i32pack": i32pack}
    return meta, l1, cores


def build(meta, repeat=1):
    """Build the SPMD Bass program (common across cores)."""
    nblk1, S1, groups1 = meta["nblk1"], meta["S1"], meta["groups1"]
    S2, nblkA, nSblk, groups2 = (meta["S2"], meta["nblkA"], meta["nSblk"],
                                 meta["groups2"])
    NROWS, Wd, nApad, TABR = (meta["NROWS"], meta["Wd"], meta["nApad"],
                              meta["TABR"])
    FW, IW32, IW16 = meta["FW"], meta["IW32"], meta["IW16"]
    oS1 = 0
    oDP1 = S1
    oM2 = S1 + nblk1
    oDP2 = oM2 + S2
    oPM = oDP2 + nblkA
    oLNN = oPM + nblkA
    oPRM = oLNN + nblkA

    nc = bacc.Bacc("TRN2", target_bir_lowering=False, debug=False,
                   num_devices=NCORES)
    dt = nc.dram_tensor
    x_in = dt("x_in", [N, D], F32, kind="ExternalInput").ap()
    wpack_in = dt("wpack_in", [D, 261], F32, kind="ExternalInput").ap()
    rows2_in = dt("rows2_in", [2, D], F32, kind="ExternalInput").ap()
    i32_in = dt("i32_in", [P, IW32], I32, kind="ExternalInput").ap()
    i16_in = dt("i16_in", [P, IW16], I16, kind="ExternalInput").ap()
    f_in = dt("f_in", [P, FW], F32, kind="ExternalInput").ap()
    out_t = dt("out", [NROWS, D], F32, kind="ExternalOutput").ap()
    tab = dt("tab", [TABR, P], F32).ap()
    scr = dt("scr", [1, D], F32).ap()

    with tile.TileContext(nc) as tc, ExitStack() as ctx:
        const = ctx.enter_context(tc.tile_pool(name="const", bufs=1))

        ident = const.tile([P, P], F32)
        make_identity(nc, ident[:])

        # ---- inputs (packed: 3 on SP, 2 on Act) ----
        i32s = const.tile([P, IW32], I32)
        nc.sync.dma_start(i32s[:], i32_in[:])
        wpk = const.tile([D, 261], F32)
        nc.sync.dma_start(wpk[:], wpack_in[:])
        W2OUTX = const.tile([D + 2, D + 2], F32)
        nc.vector.memset(W2OUTX[:, D:D + 2], 0.0)
        nc.sync.dma_start(W2OUTX[D:D + 1, 0:D], rows2_in[1:2, :])
        W2 = wpk[:, 128:192]
        nc.scalar.copy(W2OUTX[0:D, 0:D], W2)
        # warm the Exp table on Act before its first real use
        wtmp = const.tile([1, 1], F32)
        nc.vector.memset(wtmp[:], 0.0)
        wout = const.tile([1, 1], F32)
        nc.scalar.activation(wout[:], wtmp[:], AF.Exp)
        r2 = const.tile([2, D], F32)
        nc.sync.dma_start(r2[:], rows2_in[:])
        i16s = const.tile([P, IW16], I16)
        nc.sync.dma_start(i16s[:], i16_in[:])
        fs = const.tile([P, FW], F32)
        nc.sync.dma_start(fs[:], f_in[:])

        # ---- derived weights (setup PSUM pool, freed before main loop) ----
        W1T = wpk[:, 64:128]
        W2T = wpk[:, 192:256]
        av1 = wpk[:, 256:258]
        av2 = wpk[:, 258:260]
        b1col = wpk[:, 260:261]
        psc_ctx = tc.tile_pool(name="psc", bufs=1, space="PSUM")
        psc = psc_ctx.__enter__()
        # --- default-output-row chain first (gates the Pool broadcast fill) --
        defcol = const.tile([D + 2, 1], F32)
        nc.scalar.copy(defcol[0:D, :], b1col)
        nc.vector.memset(defcol[D:D + 2, :], 0.0)
        nc.vector.memset(defcol[D:D + 1, :], 1.0)
        defp = psc.tile([1, D + 2], F32, space="PSUM", tag="r1x")
        nc.tensor.matmul(defp[:], defcol[0:D + 1, :], W2OUTX[0:D + 1, :],
                         start=True, stop=True)
        defs_ = const.tile([1, D], F32)
        nc.scalar.copy(defs_[:], defp[:, 0:D])
        onesr = const.tile([1, P], F32)
        nc.vector.memset(onesr[:], 1.0)
        dbc_p = psc.tile([P, D], F32, space="PSUM", tag="bc")
        nc.tensor.matmul(dbc_p[:], onesr[:], defs_[:], start=True, stop=True)
        defbc = const.tile([P, 1, D], F32)
        nc.scalar.copy(defbc[:], dbc_p[:].rearrange("p (o f) -> p o f", o=1))
        # --- remaining derived weights ---
        wt1_p = psc.tile([D, 2], F32, space="PSUM", tag="v2")
        nc.tensor.matmul(wt1_p[:], W1T, av1, start=True, stop=True)
        wt1s = const.tile([D, 2], F32)
        nc.vector.tensor_copy(wt1s[:], wt1_p[:])
        wt2_p = psc.tile([D, 2], F32, space="PSUM", tag="v2")
        nc.tensor.matmul(wt2_p[:], W2T, av2, start=True, stop=True)
        wt2s = const.tile([D, 2], F32)
        nc.vector.tensor_copy(wt2s[:], wt2_p[:])
        wv1_p = psc.tile([1, D], F32, space="PSUM", tag="r1")
        nc.tensor.transpose(wv1_p[:], wt1s[:, 0:1], ident[0:D, 0:D])
        wv1 = const.tile([1, D], F32)
        nc.vector.tensor_copy(wv1[:], wv1_p[:])
        wv2_p = psc.tile([1, D], F32, space="PSUM", tag="r1")
        nc.tensor.transpose(wv2_p[:], wt1s[:, 1:2], ident[0:D, 0:D])
        wv2 = const.tile([1, D], F32)
        nc.vector.tensor_copy(wv2[:], wv2_p[:])
        wsb_p = psc.tile([P, D], F32, space="PSUM", tag="bc")
        nc.tensor.matmul(wsb_p[:], onesr[:], wv1[:], start=True, stop=True)
        w1srcb = const.tile([P, 1, D], F32)
        nc.vector.tensor_copy(w1srcb[:], wsb_p[:].rearrange("p (o f) -> p o f", o=1))
        wdb_p = psc.tile([P, D], F32, space="PSUM", tag="bc")
        nc.tensor.matmul(wdb_p[:], onesr[:], wv2[:], start=True, stop=True)
        w1dstb = const.tile([P, 1, D], F32)
        nc.vector.tensor_copy(w1dstb[:], wdb_p[:].rearrange("p (o f) -> p o f", o=1))
        b1w_p = psc.tile([1, 2], F32, space="PSUM", tag="s2")
        nc.tensor.matmul(b1w_p[:], b1col, wt2s[:], start=True, stop=True)
        b1ws = const.tile([1, 2], F32)
        nc.scalar.copy(b1ws[:], b1w_p[:])
        sd_p = psc.tile([P, 2], F32, space="PSUM", tag="p2")
        nc.tensor.matmul(sd_p[:], onesr[:], b1ws[:], start=True, stop=True)
        sdb = const.tile([P, 2], F32)
        nc.scalar.copy(sdb[:], sd_p[:])
        BIGMAT = const.tile([D + 1, D + 2], F32)
        nc.scalar.copy(BIGMAT[0:D, 0:D], wpk[:, 0:64])
        w1w2_p = psc.tile([D, 2], F32, space="PSUM", tag="v2")
        nc.tensor.matmul(w1w2_p[:], W1T, wt2s[:], start=True, stop=True)
        nc.scalar.copy(BIGMAT[0:D, D:D + 2], w1w2_p[:])
        nc.scalar.copy(BIGMAT[D:D + 1, 0:D], r2[0:1, :])
        nc.scalar.copy(BIGMAT[D:D + 1, D:D + 2], b1ws[:])
        b1W2_p = psc.tile([1, D], F32, space="PSUM", tag="r1")
        nc.tensor.matmul(b1W2_p[:], b1col, W2, start=True, stop=True)
        b1W2s = const.tile([1, D], F32)
        nc.scalar.copy(b1W2s[:], b1W2_p[:])
        nc.sync.dma_start(scr[:, :], b1W2s[:])
        nc.sync.dma_start(W2OUTX[D + 1:D + 2, 0:D], scr[:, :])
        psc_ctx.__exit__(None, None, None)

        # default-region staging tile: all three chunk DMAs read from the
        # same 48 columns (content identical); per-chunk gate columns are
        # written by data-gated ops to sequence the DMAs
        WdA = 35               # chunk A: outdef[0:35)   <- big[0:35)
        WdB = 50               # chunk B1: outdef[35:50) <- big[0:15)
        WdC = Wd - WdB         # chunk B2: outdef[50:98) <- big[0:48)
        big = const.tile([P, WdC, D], F32)
        nc.vector.tensor_copy(big[:, 0:WdA - 1, :],
                              defbc[:].to_broadcast((P, WdA - 1, D)))
        nc.vector.tensor_copy(big[:, WdA:WdC - 1, :],
                              defbc[:].to_broadcast((P, WdC - 1 - WdA, D)))

        outdef = out_t[nApad:NROWS, :].rearrange("(p w) f -> p w f", p=P)

        with tc.tile_pool(name="l1w", bufs=2) as l1w, \
             tc.tile_pool(name="gw", bufs=2) as gw, \
             tc.tile_pool(name="blk", bufs=3) as blk, \
             tc.tile_pool(name="ps", bufs=2, space="PSUM") as ps:
          for _rep in range(repeat):
            # ---- layer 1: gather dst and edge-src x rows directly ----
            # (single-column offsets: multi-column indirect is broken on HW)
            xd = l1w.tile([P, nblk1, D], F32, tag="xd")
            for b in range(nblk1):
                nc.gpsimd.indirect_dma_start(
                    out=xd[:, b, :], out_offset=None, in_=x_in[:, :],
                    in_offset=bass.IndirectOffsetOnAxis(
                        ap=i32s[:, S1 + b:S1 + b + 1], axis=0))
            L0 = groups1[0]["B"] * groups1[0]["L"]
            xg0 = l1w.tile([P, L0, D], F32, tag="xg0")
            for s in range(L0):
                nc.gpsimd.indirect_dma_start(
                    out=xg0[:, s, :], out_offset=None, in_=x_in[:, :],
                    in_offset=bass.IndirectOffsetOnAxis(
                        ap=i32s[:, s:s + 1], axis=0))
            xg1 = l1w.tile([P, max(S1 - L0, 1), D], F32, tag="xg1")
            for s in range(L0, S1):
                nc.gpsimd.indirect_dma_start(
                    out=xg1[:, s - L0, :], out_offset=None, in_=x_in[:, :],
                    in_offset=bass.IndirectOffsetOnAxis(
                        ap=i32s[:, s:s + 1], axis=0))
            # per-slot/per-position attention pre-activations
            t2 = l1w.tile([P, nblk1, D], F32, tag="t2")
            nc.vector.tensor_tensor(t2[:], xd[:],
                                    w1dstb[:].to_broadcast((P, nblk1, D)),
                                    op=OP.mult)
            a1d = l1w.tile([P, nblk1], F32, tag="a1d")
            nc.vector.tensor_reduce(a1d[:], t2[:], axis=mybir.AxisListType.X,
                                    op=OP.add)
            nc.vector.scalar_tensor_tensor(
                big[:, WdA - 1:WdA, :], xd[:, 0:1, :], 0.0,
                defbc[:].to_broadcast((P, 1, D)), op0=OP.mult, op1=OP.add)
            nc.sync.dma_start(outdef[:, 0:WdA, :], big[:, 0:WdA, :])
            a1s = l1w.tile([P, S1], F32, tag="a1s")
            t1 = l1w.tile([P, L0, D], F32, tag="t1")
            nc.vector.tensor_tensor(t1[:], xg0[:],
                                    w1srcb[:].to_broadcast((P, L0, D)),
                                    op=OP.mult)
            nc.vector.tensor_reduce(a1s[:, 0:L0], t1[:],
                                    axis=mybir.AxisListType.X, op=OP.add)
            if S1 > L0:
                t1b = l1w.tile([P, S1 - L0, D], F32, tag="t1b")
                nc.vector.tensor_tensor(
                    t1b[:], xg1[:, 0:S1 - L0, :],
                    w1srcb[:].to_broadcast((P, S1 - L0, D)), op=OP.mult)
                nc.vector.tensor_reduce(a1s[:, L0:S1], t1b[:],
                                        axis=mybir.AxisListType.X, op=OP.add)

            tab_sb = l1w.tile([P, nblk1, P], F32, tag="tab_sb")
            nc.vector.memset(tab_sb[:, :, D + 2:P], 0.0)
            for g in groups1:
                B, L, off = g["B"], g["L"], g["slot_off"]
                BL = B * L
                s_t = gw.tile([P, B, L], F32, tag="s_t")
                nc.vector.tensor_tensor(
                    s_t[:], a1s[:, off:off + BL],
                    a1d[:, g["b0"]:g["b0"] + B].to_broadcast((P, B, L)),
                    op=OP.add)
                u_t = gw.tile([P, B, L], F32, tag="u_t")
                nc.vector.scalar_tensor_tensor(u_t[:], s_t[:], NEG_SLOPE,
                                               s_t[:], op0=OP.mult, op1=OP.max)
                e2 = gw.tile([P, B, L], F32, tag="e2")
                nc.vector.scalar_tensor_tensor(
                    e2[:], u_t[:], BIG, fs[:, oS1 + off:oS1 + off + BL],
                    op0=OP.add, op1=OP.mult)
                mx = gw.tile([P, B], F32, tag="mx")
                nc.vector.tensor_reduce(mx[:], e2[:], axis=mybir.AxisListType.X,
                                        op=OP.max)
                dd = gw.tile([P, B, L], F32, tag="dd")
                nc.vector.tensor_tensor(dd[:], e2[:],
                                        mx[:].to_broadcast((P, B, L)),
                                        op=OP.subtract)
                ex = gw.tile([P, B, L], F32, tag="ex")
                nc.scalar.activation(ex[:], dd[:], AF.Exp)
                ssum = gw.tile([P, B], F32, tag="ssum")
                nc.vector.tensor_reduce(ssum[:], ex[:],
                                        axis=mybir.AxisListType.X, op=OP.add)
                sp = gw.tile([P, B], F32, tag="sp")
                nc.vector.tensor_scalar_add(sp[:], ssum[:], EPS)
                rs = gw.tile([P, B], F32, tag="rs")
                nc.vector.reciprocal(rs[:], sp[:])
                rsd = gw.tile([P, B], F32, tag="rsd")
                nc.vector.tensor_tensor(
                    rsd[:], rs[:], fs[:, oDP1 + g["b0"]:oDP1 + g["b0"] + B],
                    op=OP.mult)
                alpha = gw.tile([P, B, L], F32, tag="alpha")
                nc.vector.tensor_tensor(alpha[:], ex[:],
                                        rsd[:].to_broadcast((P, B, L)),
                                        op=OP.mult)
                wr = gw.tile([P, BL, D], F32, tag="wr")
                xsrc = (xg0[:, off:off + BL, :] if off < L0
                        else xg1[:, off - L0:off - L0 + BL, :])
                nc.vector.tensor_tensor(
                    wr[:], xsrc,
                    alpha[:].rearrange("p b l -> p (b l)")
                    .to_broadcast((P, BL, D)), op=OP.mult)
                msgx = gw.tile([P, B, D + 1], F32, tag="msgx")
                nc.vector.memset(msgx[:, :, D:D + 1], 1.0)
                nc.vector.tensor_reduce(
                    msgx[:, :, 0:D], wr[:].rearrange("p (b l) f -> p b f l", b=B),
                    axis=mybir.AxisListType.X, op=OP.add)
                for j in range(B):
                    b = g["b0"] + j
                    tp = ps.tile([D + 2, P], F32, space="PSUM", tag="tp")
                    nc.tensor.transpose(tp[0:D + 1, :], msgx[:, j, :], ident[:])
                    mT = blk.tile([D + 1, P], F32, tag="mT1")
                    nc.vector.tensor_copy(mT[:], tp[0:D + 1, :])
                    row_p = ps.tile([P, D + 2], F32, space="PSUM", tag="acc")
                    nc.tensor.matmul(row_p[:], mT[:], BIGMAT[:],
                                     start=True, stop=True)
                    nc.scalar.copy(tab_sb[:, b, 0:D + 2], row_p[:])

            # ---- one-DMA table write (rows partition-major) ----
            nc.sync.dma_start(
                tab[:, :].rearrange("(p w) f -> p (w f)", p=P),
                tab_sb[:].rearrange("p w f -> p (w f)"))

            # ---- layer 2 gather ----
            G = l1w.tile([P, S2, P], F32, tag="G")
            nc.gpsimd.dma_gather(G[:], tab[:, :], i16s[:, 0:8 * S2],
                                 S2 * P, S2 * P, P, single_packet=False)

            # dst delta for the special-dst blocks straight from tab_sb via
            # one-hot permutation matmuls (no DRAM round-trip)
            adst = l1w.tile([P, nblkA], F32, tag="adst")
            for sb in range(nSblk):
                ad_p = ps.tile([P, 1], F32, space="PSUM", tag="ad")
                for w in range(nblk1):
                    nc.tensor.matmul(
                        ad_p[:], fs[:, oPRM + (sb * nblk1 + w) * P:
                                    oPRM + (sb * nblk1 + w + 1) * P],
                        tab_sb[:, w, 65:66],
                        start=(w == 0), stop=(w == nblk1 - 1))
                nc.scalar.copy(adst[:, sb:sb + 1], ad_p[:])
            nc.vector.scalar_tensor_tensor(
                big[:, WdB - WdA - 1:WdB - WdA, :], adst[:, 0:1].rearrange(
                    "p (b o) -> p b o", o=1).to_broadcast((P, 1, D)), 0.0,
                defbc[:].to_broadcast((P, 1, D)), op0=OP.mult, op1=OP.add)
            nc.sync.dma_start(outdef[:, WdA:WdB, :], big[:, 0:WdB - WdA, :])

            # gate column Wd-1 on the G gather: the last chunk's transfer
            # then overlaps the layer-2 compute tail instead of delaying G
            nc.vector.scalar_tensor_tensor(
                big[:, WdC - 1:WdC, :], G[:, 0:1, 0:D], 0.0,
                defbc[:].to_broadcast((P, 1, D)), op0=OP.mult, op1=OP.add)
            nc.sync.dma_start(outdef[:, WdB:Wd, :], big[:, 0:WdC, :])

            # ---- per-position dst terms ----
            if nblkA > nSblk:
                nc.vector.tensor_copy(
                    adst[:, nSblk:nblkA],
                    sdb[:, 1:2].to_broadcast((P, nblkA - nSblk)))
            smt = l1w.tile([P, nblkA], F32, tag="smt")
            nc.vector.tensor_tensor(smt[:], adst[:],
                                    sdb[:, 0:1].to_broadcast((P, nblkA)),
                                    op=OP.add)
            ck = l1w.tile([P, nblkA], F32, tag="ck")
            nc.vector.scalar_tensor_tensor(ck[:], smt[:], NEG_SLOPE, smt[:],
                                           op0=OP.mult, op1=OP.max)
            ck2 = l1w.tile([P, nblkA], F32, tag="ck2")
            nc.vector.tensor_tensor(ck2[:], ck[:], fs[:, oLNN:oLNN + nblkA],
                                    op=OP.add)
            cb = l1w.tile([P, nblkA], F32, tag="cb")
            nc.vector.scalar_tensor_tensor(cb[:], ck2[:], BIG,
                                           fs[:, oPM:oPM + nblkA],
                                           op0=OP.add, op1=OP.mult)

            o_all = l1w.tile([P, nblkA, D], F32, tag="o_all")
            for g in groups2:
                B, L, off = g["B"], g["L"], g["slot_off"]
                BL = B * L
                b0 = g["b0"]
                asrc = G[:, off:off + BL, 64:65].rearrange("p s o -> p (s o)")
                s_t = gw.tile([P, B, L], F32, tag="s_t2")
                nc.vector.tensor_tensor(
                    s_t[:], asrc, adst[:, b0:b0 + B].to_broadcast((P, B, L)),
                    op=OP.add)
                u_t = gw.tile([P, B, L], F32, tag="u_t2")
                nc.vector.scalar_tensor_tensor(u_t[:], s_t[:], NEG_SLOPE,
                                               s_t[:], op0=OP.mult, op1=OP.max)
                e2 = gw.tile([P, B, L], F32, tag="e2b")
                nc.vector.scalar_tensor_tensor(
                    e2[:], u_t[:], BIG, fs[:, oM2 + off:oM2 + off + BL],
                    op0=OP.add, op1=OP.mult)
                mx = gw.tile([P, B], F32, tag="mxb")
                nc.vector.tensor_reduce(mx[:], e2[:], axis=mybir.AxisListType.X,
                                        op=OP.max)
                mm2 = gw.tile([P, B], F32, tag="mm2")
                nc.vector.tensor_tensor(mm2[:], mx[:], cb[:, b0:b0 + B],
                                        op=OP.max)
                dd = gw.tile([P, B, L], F32, tag="ddb")
                nc.vector.tensor_tensor(dd[:], e2[:],
                                        mm2[:].to_broadcast((P, B, L)),
                                        op=OP.subtract)
                ex = gw.tile([P, B, L], F32, tag="exb")
                nc.scalar.activation(ex[:], dd[:], AF.Exp)
                zd0 = gw.tile([P, B], F32, tag="zd0")
                nc.vector.tensor_tensor(zd0[:], cb[:, b0:b0 + B], mm2[:],
                                        op=OP.subtract)
                zd1 = gw.tile([P, B], F32, tag="zd1")
                nc.scalar.activation(zd1[:], zd0[:], AF.Exp)
                zdef = gw.tile([P, B], F32, tag="zdef")
                nc.vector.tensor_tensor(zdef[:], zd1[:],
                                        fs[:, oPM + b0:oPM + b0 + B],
                                        op=OP.mult)
                ssum = gw.tile([P, B], F32, tag="ssumb")
                nc.vector.tensor_reduce(ssum[:], ex[:],
                                        axis=mybir.AxisListType.X, op=OP.add)
                Z = gw.tile([P, B], F32, tag="Z")
                nc.vector.tensor_tensor(Z[:], ssum[:], zdef[:], op=OP.add)
                sp = gw.tile([P, B], F32, tag="spb")
                nc.vector.tensor_scalar_add(sp[:], Z[:], EPS)
                rs = gw.tile([P, B], F32, tag="rsb")
                nc.vector.reciprocal(rs[:], sp[:])
                rsd = gw.tile([P, B], F32, tag="rsdb")
                nc.vector.tensor_tensor(
                    rsd[:], rs[:], fs[:, oDP2 + b0:oDP2 + b0 + B], op=OP.mult)
                alpha = gw.tile([P, B, L], F32, tag="alphab")
                nc.vector.tensor_tensor(alpha[:], ex[:],
                                        rsd[:].to_broadcast((P, B, L)),
                                        op=OP.mult)
                msg = gw.tile([P, B, D + 2], F32, tag="msgb")
                nc.vector.memset(msg[:, :, D:D + 1], 1.0)
                wdef = gw.tile([P, B], F32, tag="wdef")
                nc.vector.tensor_tensor(wdef[:], zdef[:], rsd[:], op=OP.mult)
                nc.vector.tensor_copy(msg[:, :, D + 1:D + 2],
                                      wdef[:].rearrange("p (b o) -> p b o", o=1))
                wr = gw.tile([P, BL, D], F32, tag="wrb")
                nc.vector.tensor_tensor(
                    wr[:], G[:, off:off + BL, 0:D],
                    alpha[:].rearrange("p b l -> p (b l)")
                    .to_broadcast((P, BL, D)), op=OP.mult)
                nc.vector.tensor_reduce(
                    msg[:, :, 0:D], wr[:].rearrange("p (b l) f -> p b f l", b=B),
                    axis=mybir.AxisListType.X, op=OP.add)
                for j in range(B):
                    b = b0 + j
                    tp = ps.tile([D + 2, P], F32, space="PSUM", tag="tp")
                    nc.tensor.transpose(tp[:], msg[:, j, :], ident[:])
                    mT = blk.tile([D + 2, P], F32, tag="mT2")
                    nc.vector.tensor_copy(mT[:], tp[:])
                    o_p = ps.tile([P, D + 2], F32, space="PSUM", tag="acc")
                    nc.tensor.matmul(o_p[:], mT[:], W2OUTX[:],
                                     start=True, stop=True)
                    nc.scalar.copy(o_all[:, b, :], o_p[:, 0:D])

            # ---- one-DMA affected-region write (rows partition-major) ----
            nc.scalar.dma_start(
                out_t[0:nApad, :].rearrange("(p w) f -> p (w f)", p=P),
                o_all[:].rearrange("p w f -> p (w f)"))

    nc.compile()
    return nc


def make_in_maps(inputs, meta, l1, cores):
    x = np.ascontiguousarray(np.asarray(inputs["x"], dtype=np.float32))
    W1 = np.asarray(inputs["W1"], dtype=np.float32)
    W2 = np.asarray(inputs["W2"], dtype=np.float32)
    wpack = np.concatenate([
        W1, W1.T, W2, W2.T,
        np.stack([np.asarray(inputs["a_src1"]), np.asarray(inputs["a_dst1"])],
                 axis=1).astype(np.float32),
        np.stack([np.asarray(inputs["a_src2"]), np.asarray(inputs["a_dst2"])],
                 axis=1).astype(np.float32),
        np.asarray(inputs["b1"], dtype=np.float32).reshape(D, 1),
    ], axis=1)
    rows2 = np.stack([np.asarray(inputs["b1"], dtype=np.float32),
                      np.asarray(inputs["b2"], dtype=np.float32)])
    base = {
        "x_in": x,
        "wpack_in": np.ascontiguousarray(wpack),
        "rows2_in": np.ascontiguousarray(rows2),
        "i32_in": l1["i32pack"],
    }
    in_maps = []
    for c in range(NCORES):
        m = dict(base)
        m["i16_in"] = cores[c]["i16pack"]
        m["f_in"] = cores[c]["fpack"]
        in_maps.append(m)
    return in_maps


def unshard_core(oc, core, meta):
    nApad, nblkA = meta["nApad"], meta["nblkA"]
    A_sorted = core["A_sorted"]
    non = core["non"]
    got = np.empty((NPC, D), np.float32)
    q = np.arange(len(A_sorted))
    got[A_sorted] = oc[(q % P) * nblkA + q // P]
    got[non] = oc[nApad:nApad + len(non)]
    return got


def unshard(results, cores, meta):
    out = np.empty((N, D), np.float32)
    for c in range(NCORES):
        out[c * NPC:(c + 1) * NPC] = unshard_core(
            np.asarray(results[c]["out"]), cores[c], meta)
    return out


def kernel(**inputs):
    meta, l1, cores = prep(inputs)
    nc = build(meta, repeat=1)
    in_maps = make_in_maps(inputs, meta, l1, cores)
    res = run_bass_kernel_spmd(nc, in_maps, core_ids=list(range(NCORES)))
    return unshard(res.results, cores, meta)
